# revision 19
# baseline (speedup 1.0000x reference)
"""AttentionDecoder Trainium2 kernel (8 NeuronCores).

Strategy (v2):
  - Batch-shard everything: core c owns batches [4c, 4c+4). No collectives.
  - enc_proj = W_attn-projected encoder is precomputed once per core, and the
    embedding contribution to GRU-layer-0 gates (+ its biases) is precomputed
    for all 64 steps in one GEMM, so the per-step recurrence only contracts
    ctx/h terms.
  - The recurrence keeps every activation in [feature-on-partitions, batch]
    layout. GRU matmuls run with the weight chunk as the 128x128 stationary
    operand (bf16, fast-weight-load) and the 4-wide activations moving, so
    gates land as [gate_dim, batch] and all gate math is short-free-dim
    DVE/ACT ops. sigmoid(x) = 0.5*tanh(x/2)+0.5 keeps the scalar engine on
    one activation table (exp+tanh) forever.
  - Attention scores for the 4 batches accumulate into one PSUM bank at
    partitions {0,32,64,96}; the length mask joins the accumulation as a
    1-row matmul issued a step early; softmax skips max-subtraction and the
    1/sum normalization is folded into the ctx PSUM->SBUF copy as a
    per-partition scale.
  - Y^T = [h1; ctx] accumulates in SBUF in bf16. Phase 2 computes the full
    vocab for the local 4 batches, streaming W_out^T (bf16) from HBM in
    2 MB super-tiles that double-buffer against the GEMM.
"""

import numpy as np
import ml_dtypes

import concourse.bass as bass
import concourse.bacc as bacc_mod
import concourse.mybir as mybir
from concourse import tile
from concourse.bass_utils import run_bass_kernel_spmd

B, T, U = 32, 512, 64
V, H, E = 32000, 512, 512
NCORES = 8
BL = B // NCORES          # local batches per core
NSUP = 32                 # phase-2 vocab super-tiles (8 x 128 vocab each)
VP = NSUP * 8 * 128       # padded vocab (32768)
UB_L = U * BL             # 256 local (u, b) columns

F32 = mybir.dt.float32
F32R = mybir.dt.float32r
BF16 = mybir.dt.bfloat16
AX = mybir.AxisListType
ALU = mybir.AluOpType
ACTF = mybir.ActivationFunctionType


def build_nc(u_steps=U, biases_zero=True):
    nc = bacc_mod.Bacc()

    encE_d = nc.declare_dram_parameter("encE", [128, 4, BL, T], BF16, isOutput=False)
    encT_d = nc.declare_dram_parameter("encT", [128, 4, BL, E], BF16, isOutput=False)
    embT_d = nc.declare_dram_parameter("embT", [128, 4, U, BL], BF16, isOutput=False)
    mask_d = nc.declare_dram_parameter("mask", [1, BL, T], BF16, isOutput=False)
    wattnT_d = nc.declare_dram_parameter("wattnT", [128, 4, 4, 128], BF16, isOutput=False)
    wemb0_d = nc.declare_dram_parameter("wemb0", [128, 4, 12, 128], BF16, isOutput=False)
    wrz0_d = nc.declare_dram_parameter("wrz0", [128, 8, 8, 128], BF16, isOutput=False)
    wn0i_d = nc.declare_dram_parameter("wn0i", [128, 4, 4, 128], BF16, isOutput=False)
    wn0h_d = nc.declare_dram_parameter("wn0h", [128, 4, 4, 128], BF16, isOutput=False)
    wrz1_d = nc.declare_dram_parameter("wrz1", [128, 8, 8, 128], BF16, isOutput=False)
    wn1i_d = nc.declare_dram_parameter("wn1i", [128, 4, 4, 128], BF16, isOutput=False)
    wn1h_d = nc.declare_dram_parameter("wn1h", [128, 4, 4, 128], BF16, isOutput=False)
    woutT_d = nc.declare_dram_parameter("woutT", [NSUP, 128, 8, 8, 128], BF16, isOutput=False)
    bout_d = nc.declare_dram_parameter("bout", [128, NSUP * 8], F32, isOutput=False)
    identb_d = nc.declare_dram_parameter("identb", [128, 128], BF16, isOutput=False)
    bias0_d = nc.declare_dram_parameter("bias0", [128, 12], F32, isOutput=False)
    brz1_d = nc.declare_dram_parameter("brz1", [128, 8, BL], F32, isOutput=False)
    bnh0_d = nc.declare_dram_parameter("bnh0", [128, 4, BL], F32, isOutput=False)
    bni1_d = nc.declare_dram_parameter("bni1", [128, 4, BL], F32, isOutput=False)
    bnh1_d = nc.declare_dram_parameter("bnh1", [128, 4, BL], F32, isOutput=False)
    out_d = nc.declare_dram_parameter("out", [NSUP, 128, 8, U, BL], BF16, isOutput=True)

    with tile.TileContext(nc) as tc:
        with tc.tile_pool(name="res", bufs=1) as res:
            # ---- resident SBUF ----
            encT_sb = res.tile([128, 4, BL, E], BF16, tag="encT")
            nc.sync.dma_start(encT_sb[:], encT_d.ap())
            embT_sb = res.tile([128, 4, U, BL], BF16, tag="embT")
            nc.sync.dma_start(embT_sb[:], embT_d.ap())
            mask_sb = res.tile([1, BL, T], BF16, tag="mask")
            nc.sync.dma_start(mask_sb[:], mask_d.ap())
            wrz0_sb = res.tile([128, 8, 8, 128], BF16, tag="wrz0")
            nc.sync.dma_start(wrz0_sb[:], wrz0_d.ap())
            wn0i_sb = res.tile([128, 4, 4, 128], BF16, tag="wn0i")
            nc.sync.dma_start(wn0i_sb[:], wn0i_d.ap())
            wn0h_sb = res.tile([128, 4, 4, 128], BF16, tag="wn0h")
            nc.sync.dma_start(wn0h_sb[:], wn0h_d.ap())
            wrz1_sb = res.tile([128, 8, 8, 128], BF16, tag="wrz1")
            nc.sync.dma_start(wrz1_sb[:], wrz1_d.ap())
            wn1i_sb = res.tile([128, 4, 4, 128], BF16, tag="wn1i")
            nc.sync.dma_start(wn1i_sb[:], wn1i_d.ap())
            wn1h_sb = res.tile([128, 4, 4, 128], BF16, tag="wn1h")
            nc.sync.dma_start(wn1h_sb[:], wn1h_d.ap())
            identb_sb = res.tile([128, 128], BF16, tag="identb")
            nc.sync.dma_start(identb_sb[:], identb_d.ap())
            bout_sb = res.tile([128, NSUP * 8], F32, tag="bout")
            nc.sync.dma_start(bout_sb[:], bout_d.ap())
            if not biases_zero:
                bias0_sb = res.tile([128, 12], F32, tag="bias0")
                nc.sync.dma_start(bias0_sb[:], bias0_d.ap())
                brz1_sb = res.tile([128, 8, BL], F32, tag="brz1")
                nc.sync.dma_start(brz1_sb[:], brz1_d.ap())
                bnh0_sb = res.tile([128, 4, BL], F32, tag="bnh0")
                nc.sync.dma_start(bnh0_sb[:], bnh0_d.ap())
                bni1_sb = res.tile([128, 4, BL], F32, tag="bni1")
                nc.sync.dma_start(bni1_sb[:], bni1_d.ap())
                bnh1_sb = res.tile([128, 4, BL], F32, tag="bnh1")
                nc.sync.dma_start(bnh1_sb[:], bnh1_d.ap())

            NPRE = 3
            wpre_sb = [res.tile([128, 8, 8, 128], BF16, tag=f"wpre{i}",
                                name=f"wpre{i}")
                       for i in range(NPRE)]
            for i in range(NPRE):
                nc.sync.dma_start(wpre_sb[i][:], woutT_d.ap()[i])

            encP_sb = res.tile([128, 4, BL, T], BF16, tag="encP")
            embW0_sb = res.tile([128, 12, U, BL], F32, tag="embW0")
            yT_sb = res.tile([128, 8, U, BL], BF16, tag="yT")

            # persistent recurrence state (h kept in bf16)
            h0b = res.tile([128, 4, BL], BF16, tag="h0b")
            zero_sb = res.tile([128, 4, BL], BF16, tag="zero")
            nc.gpsimd.memset(h0b[:], 0.0)
            nc.gpsimd.memset(zero_sb[:], 0.0)

            # ---- setup: encP = W_attn^T-projected enc; embW0 = Wih0_emb @ emb ----
            with (
                tc.tile_pool(name="su", bufs=1) as su,
                tc.tile_pool(name="sup", bufs=2, space="PSUM") as sup,
            ):
                encE_sb = su.tile([128, 4, BL, T], BF16, tag="encE")
                nc.sync.dma_start(encE_sb[:], encE_d.ap())
                wattnT_sb = su.tile([128, 4, 4, 128], BF16, tag="wattnT")
                nc.sync.dma_start(wattnT_sb[:], wattnT_d.ap())
                wemb0_sb = su.tile([128, 4, 12, 128], BF16, tag="wemb0")
                nc.sync.dma_start(wemb0_sb[:], wemb0_d.ap())

                for m in range(12):
                    ew_ps = sup.tile([128, U * BL], F32, tag="ewps")
                    for kc in range(4):
                        nc.tensor.matmul(
                            ew_ps[:],
                            wemb0_sb[:, kc, m, :],
                            embT_sb[:, kc, :, :],
                            start=(kc == 0), stop=(kc == 3),
                        )
                    if biases_zero:
                        if m % 2 == 0:
                            nc.vector.tensor_copy(embW0_sb[:, m, :, :], ew_ps[:])
                        else:
                            nc.scalar.copy(embW0_sb[:, m, :, :], ew_ps[:])
                    else:
                        nc.scalar.activation(embW0_sb[:, m, :, :], ew_ps[:],
                                             ACTF.Identity, bias=bias0_sb[:, m:m + 1])

                for b in range(BL):
                    for hc in range(4):
                        ep_ps = sup.tile([128, T], F32, tag="epps")
                        for ec in range(4):
                            nc.tensor.matmul(
                                ep_ps[:],
                                wattnT_sb[:, ec, hc, :],
                                encE_sb[:, ec, b, :],
                                start=(ec == 0), stop=(ec == 3),
                            )
                        if (b + hc) % 2 == 0:
                            nc.vector.tensor_copy(encP_sb[:, hc, b, :], ep_ps[:])
                        else:
                            nc.scalar.copy(encP_sb[:, hc, b, :], ep_ps[:])

            # ---- phase 1: recurrence ----
            with (
                tc.tile_pool(name="p1s", bufs=1) as p1s,
                tc.tile_pool(name="p1p", bufs=1, space="PSUM") as p1p,
            ):
                # persistent PSUM tiles; scores rotate on u parity for the
                # mask pre-accumulation trick
                sc_ps = [p1p.tile([128, T], F32, tag=f"sc{i}", name=f"sc{i}")
                         for i in range(2)]
                tT_ps = p1p.tile([128, 8, 128], BF16, tag="tT")
                ctx_ps = p1p.tile([128, E], F32, tag="ctx")
                rz0_ps = p1p.tile([128, 8, BL], F32, tag="rz0")
                nih0_ps = p1p.tile([128, 8, BL], F32, tag="nih0")
                rz1_ps = p1p.tile([128, 8, BL], F32, tag="rz1")
                nih1_ps = p1p.tile([128, 8, BL], F32, tag="nih1")

                att_sb = p1s.tile([128, T], BF16, tag="att")
                ssum = p1s.tile([128, 1], F32, tag="ssum")
                rec = p1s.tile([128, 1], F32, tag="rec")
                atT_sb = p1s.tile([128, 4, BL], BF16, tag="atTsb")
                ctx_sb = p1s.tile([128, E], BF16, tag="ctxsb")
                g0_sb = p1s.tile([128, 8, BL], F32, tag="g0")
                t0_sb = p1s.tile([128, 8, BL], F32, tag="t0")
                ni0_sb = p1s.tile([128, 4, BL], F32, tag="ni0")
                a0_sb = p1s.tile([128, 4, BL], F32, tag="a0")
                np0_sb = p1s.tile([128, 4, BL], F32, tag="np0")
                n0_sb = p1s.tile([128, 4, BL], F32, tag="n0")
                d0_sb = p1s.tile([128, 4, BL], F32, tag="d0")
                g1_sb = p1s.tile([128, 8, BL], F32, tag="g1")
                t1_sb = p1s.tile([128, 8, BL], F32, tag="t1")
                a1_sb = p1s.tile([128, 4, BL], F32, tag="a1")
                np1_sb = p1s.tile([128, 4, BL], F32, tag="np1")
                n1_sb = p1s.tile([128, 4, BL], F32, tag="n1")
                d1_sb = p1s.tile([128, 4, BL], F32, tag="d1")
                ones_sb = p1s.tile([1, 1], BF16, tag="ones")
                ones128_sb = p1s.tile([1, 128], BF16, tag="ones128")
                zrow_sb = p1s.tile([1, T], BF16, tag="zrow")
                nc.gpsimd.memset(ones_sb[:], 1.0)
                nc.gpsimd.memset(ones128_sb[:], 1.0)
                nc.gpsimd.memset(zrow_sb[:], 0.0)

                # one-time init: write every partition row of the score/ctx
                # banks so never-again-written rows hold 0, not pre-kernel
                # garbage (exp/transpose would otherwise see inf/NaN there).
                for i in range(2):
                    nc.tensor.matmul(
                        sc_ps[i][:, :], ones128_sb[:], zrow_sb[:],
                        start=True, stop=False, skip_group_check=True,
                    )
                nc.tensor.matmul(
                    ctx_ps[:, :], ones128_sb[:], zrow_sb[:, 0:E],
                    start=True, stop=False, skip_group_check=True,
                )

                # mask pre-accumulation for u=0
                for b in range(BL):
                    nc.tensor.matmul(
                        sc_ps[0][32 * b:32 * b + 1, :],
                        ones_sb[:], mask_sb[:, b, :],
                        start=True, stop=False, skip_group_check=True,
                        tile_position=(0, 32 * b),
                    )

                def mm_nh(gps, wsb, rhs_fn, u):
                    for m in range(4):
                        for k in range(4):
                            nc.tensor.matmul(
                                gps[:, 4 + m, :], wsb[:, k, m, :], rhs_fn(k),
                                start=(k == 0), stop=(k == 3),
                            )

                def mm_rz_h(gps, wsb, rhs_fn, u, ms=range(8)):
                    for m in ms:
                        for k in range(4):
                            nc.tensor.matmul(
                                gps[:, m, :], wsb[:, k, m, :], rhs_fn(k),
                                start=(k == 0), stop=False,
                            )

                def h0rhs_fn(u):
                    return (lambda k: zero_sb[:, k, :]) if u == 0 else \
                           (lambda k: h0b[:, k, :])

                def h1rhs_fn(u):
                    return (lambda k: zero_sb[:, k, :]) if u == 0 else \
                           (lambda k: yT_sb[:, k, u - 1, :])

                # u=0 h-dependent GRU0 contractions (zeros)
                mm_nh(nih0_ps, wn0h_sb, h0rhs_fn(0), 0)
                mm_rz_h(rz0_ps, wrz0_sb, h0rhs_fn(0), 0)

                for u in range(u_steps):
                    cur = sc_ps[u % 2]
                    nxt = sc_ps[(u + 1) % 2]

                    # scores[b, t] += sum_h q[h, b] * encP[b][h, t]
                    for b in range(BL):
                        for kc in range(4):
                            lhs = (embT_sb[:, kc, 0, b:b + 1] if u == 0
                                   else yT_sb[:, kc, u - 1, b:b + 1])
                            nc.tensor.matmul(
                                cur[32 * b:32 * b + 1, :],
                                lhs,
                                encP_sb[:, kc, b, :],
                                start=False, stop=(kc == 3),
                                skip_group_check=True,
                                tile_position=(0, 32 * b),
                            )

                    # GRU1 h1-dependent contractions fill the softmax gap
                    mm_nh(nih1_ps, wn1h_sb, h1rhs_fn(u), u)
                    mm_rz_h(rz1_ps, wrz1_sb, h1rhs_fn(u), u, ms=range(4))

                    # softmax (no max-subtract; mask rows are -1e30)
                    nc.scalar.activation(att_sb[:], cur[:], ACTF.Exp,
                                         accum_out=ssum[:])
                    nc.vector.reciprocal(rec[:], ssum[:])

                    # attT: batch b sits in column 32b; keep those columns
                    for tc4 in range(4):
                        nc.tensor.transpose(
                            tT_ps[:, tc4, :],
                            att_sb[:, tc4 * 128:(tc4 + 1) * 128],
                            identb_sb[:],
                        )
                    nc.vector.tensor_copy(atT_sb[:], tT_ps[:, 0:4, 0:128:32])

                    # ctx[b, e] += att[b, t] * encT[b][t, e]
                    for b in range(BL):
                        for tc4 in range(4):
                            nc.tensor.matmul(
                                ctx_ps[32 * b:32 * b + 1, :],
                                atT_sb[:, tc4, b:b + 1],
                                encT_sb[:, tc4, b, :],
                                start=(tc4 == 0), stop=(tc4 == 3),
                                skip_group_check=True,
                                tile_position=(0, 32 * b),
                            )

                    # ctx normalize-on-copy (scale = 1/sum per batch row),
                    # split across scalar+vector engines
                    nc.vector.tensor_scalar_mul(ctx_sb[:, 0:E // 2],
                                                ctx_ps[:, 0:E // 2], rec[:])
                    nc.scalar.activation(ctx_sb[:, E // 2:E],
                                         ctx_ps[:, E // 2:E], ACTF.Copy,
                                         scale=rec[:])
                    # ctxT transposes then GRU0 ctx contractions
                    for ec in range(4):
                        nc.tensor.transpose(
                            tT_ps[:, 4 + ec, :],
                            ctx_sb[:, ec * 128:(ec + 1) * 128],
                            identb_sb[:],
                        )
                    nc.vector.tensor_copy(yT_sb[:, 4:8, u, :],
                                          tT_ps[:, 4:8, 0:128:32])
                    for m in range(8):
                        for k in range(4):
                            nc.tensor.matmul(
                                rz0_ps[:, m, :],
                                wrz0_sb[:, 4 + k, m, :],
                                yT_sb[:, 4 + k, u, :],
                                start=False, stop=(k == 3),
                            )
                    for m in range(4):
                        for k in range(4):
                            nc.tensor.matmul(
                                nih0_ps[:, m, :],
                                wn0i_sb[:, k, m, :],
                                yT_sb[:, 4 + k, u, :],
                                start=(k == 0), stop=(k == 3),
                            )

                    # mask pre-accumulation for u+1 fills the gate0 gap
                    if u + 1 < u_steps:
                        for b in range(BL):
                            nc.tensor.matmul(
                                nxt[32 * b:32 * b + 1, :],
                                ones_sb[:], mask_sb[:, b, :],
                                start=True, stop=False, skip_group_check=True,
                                tile_position=(0, 32 * b),
                            )

                    # ---- GRU0 gate math ([128, m, b] layout) ----
                    nc.vector.tensor_tensor(g0_sb[:], rz0_ps[:],
                                            embW0_sb[:, 0:8, u, :], op=ALU.add)
                    nc.scalar.activation(t0_sb[:], g0_sb[:], ACTF.Tanh,
                                         scale=0.5)
                    nc.vector.tensor_tensor(ni0_sb[:], nih0_ps[:, 0:4, :],
                                            embW0_sb[:, 8:12, u, :], op=ALU.add)
                    if biases_zero:
                        nc.vector.scalar_tensor_tensor(
                            a0_sb[:], t0_sb[:, 0:4, :], 1.0,
                            nih0_ps[:, 4:8, :],
                            op0=ALU.add, op1=ALU.mult)
                    else:
                        nc.vector.tensor_tensor(a0_sb[:],
                                                nih0_ps[:, 4:8, :],
                                                bnh0_sb[:], op=ALU.add)
                        nc.vector.scalar_tensor_tensor(
                            a0_sb[:], t0_sb[:, 0:4, :], 1.0, a0_sb[:],
                            op0=ALU.add, op1=ALU.mult)
                    nc.vector.scalar_tensor_tensor(
                        np0_sb[:], a0_sb[:], 0.5, ni0_sb[:],
                        op0=ALU.mult, op1=ALU.add)
                    nc.scalar.activation(n0_sb[:], np0_sb[:], ACTF.Tanh)
                    nc.vector.tensor_tensor(d0_sb[:], h0b[:], n0_sb[:],
                                            op=ALU.subtract)
                    nc.vector.scalar_tensor_tensor(
                        d0_sb[:], t0_sb[:, 4:8, :], 1.0, d0_sb[:],
                        op0=ALU.add, op1=ALU.mult)
                    nc.vector.scalar_tensor_tensor(
                        h0b[:], d0_sb[:], 0.5, n0_sb[:],
                        op0=ALU.mult, op1=ALU.add)

                    # GRU1 h0n-dependent contractions
                    for m in range(8):
                        for k in range(4):
                            nc.tensor.matmul(
                                rz1_ps[:, m, :],
                                wrz1_sb[:, 4 + k, m, :],
                                h0b[:, k, :],
                                start=False, stop=(k == 3),
                            )
                    for m in range(4):
                        for k in range(4):
                            nc.tensor.matmul(
                                nih1_ps[:, m, :],
                                wn1i_sb[:, k, m, :],
                                h0b[:, k, :],
                                start=(k == 0), stop=(k == 3),
                            )

                    # next step's h0-dependent GRU0 contractions fill the
                    # GRU1 gate-math gap
                    if u + 1 < u_steps:
                        mm_nh(nih0_ps, wn0h_sb, h0rhs_fn(u + 1), u + 1)
                        mm_rz_h(rz0_ps, wrz0_sb, h0rhs_fn(u + 1), u + 1)

                    # ---- GRU1 gate math ----
                    if biases_zero:
                        nc.scalar.activation(t1_sb[:], rz1_ps[:],
                                             ACTF.Tanh, scale=0.5)
                        nc.vector.scalar_tensor_tensor(
                            a1_sb[:], t1_sb[:, 0:4, :], 1.0,
                            nih1_ps[:, 4:8, :],
                            op0=ALU.add, op1=ALU.mult)
                        nc.vector.scalar_tensor_tensor(
                            np1_sb[:], a1_sb[:], 0.5, nih1_ps[:, 0:4, :],
                            op0=ALU.mult, op1=ALU.add)
                    else:
                        nc.vector.tensor_tensor(g1_sb[:], rz1_ps[:],
                                                brz1_sb[:], op=ALU.add)
                        nc.scalar.activation(t1_sb[:], g1_sb[:], ACTF.Tanh,
                                             scale=0.5)
                        nc.vector.tensor_tensor(a1_sb[:],
                                                nih1_ps[:, 4:8, :],
                                                bnh1_sb[:], op=ALU.add)
                        nc.vector.scalar_tensor_tensor(
                            a1_sb[:], t1_sb[:, 0:4, :], 1.0, a1_sb[:],
                            op0=ALU.add, op1=ALU.mult)
                        nc.vector.tensor_tensor(np1_sb[:],
                                                nih1_ps[:, 0:4, :],
                                                bni1_sb[:], op=ALU.add)
                        nc.vector.scalar_tensor_tensor(
                            np1_sb[:], a1_sb[:], 0.5, np1_sb[:],
                            op0=ALU.mult, op1=ALU.add)
                    nc.scalar.activation(n1_sb[:], np1_sb[:], ACTF.Tanh)
                    d1_rhs = (zero_sb[:, :, :] if u == 0
                              else yT_sb[:, 0:4, u - 1, :])
                    nc.vector.tensor_tensor(d1_sb[:], d1_rhs, n1_sb[:],
                                            op=ALU.subtract)
                    nc.vector.scalar_tensor_tensor(
                        d1_sb[:], t1_sb[:, 4:8, :], 1.0, d1_sb[:],
                        op0=ALU.add, op1=ALU.mult)
                    nc.vector.scalar_tensor_tensor(
                        yT_sb[:, 0:4, u, :], d1_sb[:], 0.5, n1_sb[:],
                        op0=ALU.mult, op1=ALU.add)

            # ---- phase 2: full-vocab projection for the local batches ----
            with (
                tc.tile_pool(name="p2w", bufs=2) as p2w,
                tc.tile_pool(name="p2o", bufs=2) as p2o,
                tc.tile_pool(name="p2p", bufs=4, space="PSUM") as p2p,
            ):
                for s in range(NSUP):
                    if s < NPRE:
                        wt = wpre_sb[s]
                    else:
                        wt = p2w.tile([128, 8, 8, 128], BF16, tag="wt")
                        nc.sync.dma_start(wt[:], woutT_d.ap()[s])
                    ob = p2o.tile([128, 8, UB_L], BF16, tag="ob")
                    for vc in range(8):
                        ps = p2p.tile([128, UB_L], F32, tag="p2")
                        for kc in range(8):
                            nc.tensor.matmul(
                                ps[:],
                                wt[:, vc, kc, :],
                                yT_sb[:, kc, :, :],
                                start=(kc == 0), stop=(kc == 7),
                            )
                        if vc % 2 == 0:
                            nc.scalar.activation(
                                ob[:, vc, :], ps[:], ACTF.Identity,
                                bias=bout_sb[:, s * 8 + vc:s * 8 + vc + 1])
                        else:
                            nc.vector.tensor_scalar_add(
                                ob[:, vc, :], ps[:],
                                bout_sb[:, s * 8 + vc:s * 8 + vc + 1])
                    nc.gpsimd.dma_start(out_d.ap()[s], ob[:])

    nc.finalize()
    return nc


_NC_CACHE = {}


def _get_nc(biases_zero=True):
    if biases_zero not in _NC_CACHE:
        _NC_CACHE[biases_zero] = build_nc(biases_zero=biases_zero)
    return _NC_CACHE[biases_zero]


def make_in_maps(inputs):
    f32 = np.float32
    bf = ml_dtypes.bfloat16
    enc = np.asarray(inputs["encoder_out"], f32)
    lens = np.asarray(inputs["encoder_lens"]).astype(np.int64)
    dec = np.asarray(inputs["decoder_in"]).astype(np.int64)
    emb_table = np.asarray(inputs["emb_table"], f32)
    W_attn = np.asarray(inputs["W_attn"], f32)
    W_ih0 = np.asarray(inputs["W_ih0"], f32)
    W_hh0 = np.asarray(inputs["W_hh0"], f32)
    b_ih0 = np.asarray(inputs["b_ih0"], f32)
    b_hh0 = np.asarray(inputs["b_hh0"], f32)
    W_ih1 = np.asarray(inputs["W_ih1"], f32)
    W_hh1 = np.asarray(inputs["W_hh1"], f32)
    b_ih1 = np.asarray(inputs["b_ih1"], f32)
    b_hh1 = np.asarray(inputs["b_hh1"], f32)
    W_out = np.asarray(inputs["W_out"], f32)
    b_out = np.asarray(inputs["b_out"], f32)

    embedded = emb_table[dec]                       # [B, U, H]
    mask = np.where(
        np.arange(T)[None, :] >= lens[:, None],
        f32(-1e30), f32(0.0))                       # [B, T]

    def chunkT(w):
        # [K, M] weight -> lhsT chunks [128, kc, mc, 128] (bf16)
        K, M = w.shape
        return np.ascontiguousarray(
            w.reshape(K // 128, 128, M // 128, 128).transpose(1, 0, 2, 3)
        ).astype(bf)

    # per-step GRU lhsT chunk tables; k-order: h-part first, then ctx/x-part
    wrz0 = np.concatenate([W_hh0[0:1024].T, W_ih0[0:1024, 512:1024].T], 0)
    wrz0 = chunkT(wrz0)                             # [128, 8, 8, 128]
    wn0i = chunkT(W_ih0[1024:1536, 512:1024].T)
    wn0h = chunkT(W_hh0[1024:1536].T)
    wrz1 = np.concatenate([W_hh1[0:1024].T, W_ih1[0:1024].T], 0)
    wrz1 = chunkT(wrz1)
    wn1i = chunkT(W_ih1[1024:1536].T)
    wn1h = chunkT(W_hh1[1024:1536].T)
    wemb0 = chunkT(W_ih0[:, 0:512].T)               # [128, 4, 12, 128]
    wattnT = chunkT(W_attn.T)                       # [128, 4ec, 4hc, 128]

    Wp = np.zeros((VP, 1024), f32)
    Wp[:V] = W_out
    woutT = np.ascontiguousarray(
        Wp.reshape(NSUP, 8, 128, 8, 128).transpose(0, 4, 1, 3, 2)
    ).astype(bf)                                    # [32, 128k, 8vc, 8kc, 128v]
    bp = np.zeros((VP,), f32)
    bp[:V] = b_out
    bout_t = np.ascontiguousarray(bp.reshape(NSUP * 8, 128).T)

    # biases
    bias0 = np.zeros((128, 12), f32)                # embW0 bias (rz: ih+hh, n_i: ih)
    brz = (b_ih0[:1024] + b_hh0[:1024]).reshape(8, 128).T
    bias0[:, 0:8] = brz
    bias0[:, 8:12] = b_ih0[1024:1536].reshape(4, 128).T
    bcast = lambda v: np.ascontiguousarray(np.broadcast_to(
        v.reshape(v.shape[0] // 128, 128).T[:, :, None], (128, v.shape[0] // 128, BL)))
    brz1 = bcast(b_ih1[:1024] + b_hh1[:1024])
    bnh0 = bcast(b_hh0[1024:1536])
    bni1 = bcast(b_ih1[1024:1536])
    bnh1 = bcast(b_hh1[1024:1536])

    identb = np.eye(128, dtype=f32).astype(bf)

    in_maps = []
    for c in range(NCORES):
        bs = slice(BL * c, BL * (c + 1))
        encl = enc[bs]                              # [BL, T, E]
        encE = np.ascontiguousarray(
            encl.transpose(2, 0, 1).reshape(4, 128, BL, T).transpose(1, 0, 2, 3)
        ).astype(bf)                                # [128, 4ec, BL, T]
        encTt = np.ascontiguousarray(
            encl.transpose(1, 0, 2).reshape(4, 128, BL, E).transpose(1, 0, 2, 3)
        ).astype(bf)                                # [128, 4tc, BL, E]
        embT = np.ascontiguousarray(
            embedded[bs].transpose(2, 1, 0).reshape(4, 128, U, BL).transpose(1, 0, 2, 3)
        ).astype(bf)                                # [128, 4hc, U, BL]
        in_maps.append({
            "encE": encE,
            "encT": encTt,
            "embT": embT,
            "mask": np.ascontiguousarray(mask[bs][None, :, :]).astype(bf),
            "wattnT": wattnT,
            "wemb0": wemb0,
            "wrz0": wrz0, "wn0i": wn0i, "wn0h": wn0h,
            "wrz1": wrz1, "wn1i": wn1i, "wn1h": wn1h,
            "woutT": woutT,
            "bout": bout_t,
            "identb": identb,
            "bias0": bias0,
            "brz1": brz1, "bnh0": bnh0, "bni1": bni1, "bnh1": bnh1,
        })
    return in_maps


def assemble_output(results):
    logits = np.zeros((B, U, V), np.float32)
    for c in range(NCORES):
        o = np.asarray(results[c]["out"], np.float32)  # [32, 128, 8, U, BL]
        o = o.transpose(4, 3, 0, 2, 1).reshape(BL, U, VP)
        logits[BL * c:BL * (c + 1)] = o[:, :, :V]
    return logits


def kernel(**inputs):
    bz = all(
        float(np.abs(np.asarray(inputs[k])).max()) == 0.0
        for k in ("b_ih0", "b_hh0", "b_ih1", "b_hh1")
    )
    nc = _get_nc(biases_zero=bz)
    in_maps = make_in_maps(inputs)
    res = run_bass_kernel_spmd(nc, in_maps, core_ids=list(range(NCORES)))
    return assemble_output(res.results)


if __name__ == "__main__":
    nc = build_nc()
    print("built OK")


# revision 20
# speedup vs baseline: 1.0188x; 1.0188x over previous
"""AttentionDecoder Trainium2 kernel (8 NeuronCores).

Strategy (v2):
  - Batch-shard everything: core c owns batches [4c, 4c+4). No collectives.
  - enc_proj = W_attn-projected encoder is precomputed once per core, and the
    embedding contribution to GRU-layer-0 gates (+ its biases) is precomputed
    for all 64 steps in one GEMM, so the per-step recurrence only contracts
    ctx/h terms.
  - The recurrence keeps every activation in [feature-on-partitions, batch]
    layout. GRU matmuls run with the weight chunk as the 128x128 stationary
    operand (bf16, fast-weight-load) and the 4-wide activations moving, so
    gates land as [gate_dim, batch] and all gate math is short-free-dim
    DVE/ACT ops. sigmoid(x) = 0.5*tanh(x/2)+0.5 keeps the scalar engine on
    one activation table (exp+tanh) forever.
  - Attention scores for the 4 batches accumulate into one PSUM bank at
    partitions {0,32,64,96}; the length mask joins the accumulation as a
    1-row matmul issued a step early; softmax skips max-subtraction and the
    1/sum normalization is folded into the ctx PSUM->SBUF copy as a
    per-partition scale.
  - Y^T = [h1; ctx] accumulates in SBUF in bf16. Phase 2 computes the full
    vocab for the local 4 batches, streaming W_out^T (bf16) from HBM in
    2 MB super-tiles that double-buffer against the GEMM.
"""

import numpy as np
import ml_dtypes

import concourse.bass as bass
import concourse.bacc as bacc_mod
import concourse.mybir as mybir
from concourse import tile
from concourse.bass_utils import run_bass_kernel_spmd

B, T, U = 32, 512, 64
V, H, E = 32000, 512, 512
NCORES = 8
BL = B // NCORES          # local batches per core
NSUP = 32                 # phase-2 vocab super-tiles (8 x 128 vocab each)
VP = NSUP * 8 * 128       # padded vocab (32768)
UB_L = U * BL             # 256 local (u, b) columns

F32 = mybir.dt.float32
F32R = mybir.dt.float32r
BF16 = mybir.dt.bfloat16
AX = mybir.AxisListType
ALU = mybir.AluOpType
ACTF = mybir.ActivationFunctionType


def build_nc(u_steps=U, biases_zero=True):
    nc = bacc_mod.Bacc()

    encE_d = nc.declare_dram_parameter("encE", [128, 4, BL, T], BF16, isOutput=False)
    encT_d = nc.declare_dram_parameter("encT", [128, 4, BL, E], BF16, isOutput=False)
    embT_d = nc.declare_dram_parameter("embT", [128, 4, U, BL], BF16, isOutput=False)
    mask_d = nc.declare_dram_parameter("mask", [1, BL, T], BF16, isOutput=False)
    wattnT_d = nc.declare_dram_parameter("wattnT", [128, 4, 4, 128], BF16, isOutput=False)
    wemb0_d = nc.declare_dram_parameter("wemb0", [128, 4, 12, 128], BF16, isOutput=False)
    wrz0_d = nc.declare_dram_parameter("wrz0", [128, 8, 8, 128], BF16, isOutput=False)
    wn0i_d = nc.declare_dram_parameter("wn0i", [128, 4, 4, 128], BF16, isOutput=False)
    wn0h_d = nc.declare_dram_parameter("wn0h", [128, 4, 4, 128], BF16, isOutput=False)
    wrz1_d = nc.declare_dram_parameter("wrz1", [128, 8, 8, 128], BF16, isOutput=False)
    wn1i_d = nc.declare_dram_parameter("wn1i", [128, 4, 4, 128], BF16, isOutput=False)
    wn1h_d = nc.declare_dram_parameter("wn1h", [128, 4, 4, 128], BF16, isOutput=False)
    woutT_d = nc.declare_dram_parameter("woutT", [NSUP, 128, 8, 8, 128], BF16, isOutput=False)
    bout_d = nc.declare_dram_parameter("bout", [128, NSUP * 8], F32, isOutput=False)
    identb_d = nc.declare_dram_parameter("identb", [128, 128], BF16, isOutput=False)
    bias0_d = nc.declare_dram_parameter("bias0", [128, 12], F32, isOutput=False)
    brz1_d = nc.declare_dram_parameter("brz1", [128, 8, BL], F32, isOutput=False)
    bnh0_d = nc.declare_dram_parameter("bnh0", [128, 4, BL], F32, isOutput=False)
    bni1_d = nc.declare_dram_parameter("bni1", [128, 4, BL], F32, isOutput=False)
    bnh1_d = nc.declare_dram_parameter("bnh1", [128, 4, BL], F32, isOutput=False)
    out_d = nc.declare_dram_parameter("out", [NSUP, 128, 8, U, BL], BF16, isOutput=True)

    with tile.TileContext(nc) as tc:
        with tc.tile_pool(name="res", bufs=1) as res:
            # ---- resident SBUF ----
            encT_sb = res.tile([128, 4, BL, E], BF16, tag="encT")
            nc.sync.dma_start(encT_sb[:], encT_d.ap())
            embT_sb = res.tile([128, 4, U, BL], BF16, tag="embT")
            nc.sync.dma_start(embT_sb[:], embT_d.ap())
            mask_sb = res.tile([1, BL, T], BF16, tag="mask")
            nc.sync.dma_start(mask_sb[:], mask_d.ap())
            wrz0_sb = res.tile([128, 8, 8, 128], BF16, tag="wrz0")
            nc.sync.dma_start(wrz0_sb[:], wrz0_d.ap())
            wn0i_sb = res.tile([128, 4, 4, 128], BF16, tag="wn0i")
            nc.sync.dma_start(wn0i_sb[:], wn0i_d.ap())
            wn0h_sb = res.tile([128, 4, 4, 128], BF16, tag="wn0h")
            nc.sync.dma_start(wn0h_sb[:], wn0h_d.ap())
            wrz1_sb = res.tile([128, 8, 8, 128], BF16, tag="wrz1")
            nc.sync.dma_start(wrz1_sb[:], wrz1_d.ap())
            wn1i_sb = res.tile([128, 4, 4, 128], BF16, tag="wn1i")
            nc.sync.dma_start(wn1i_sb[:], wn1i_d.ap())
            wn1h_sb = res.tile([128, 4, 4, 128], BF16, tag="wn1h")
            nc.sync.dma_start(wn1h_sb[:], wn1h_d.ap())
            identb_sb = res.tile([128, 128], BF16, tag="identb")
            nc.sync.dma_start(identb_sb[:], identb_d.ap())
            bout_sb = res.tile([128, NSUP * 8], F32, tag="bout")
            nc.sync.dma_start(bout_sb[:], bout_d.ap())
            if not biases_zero:
                bias0_sb = res.tile([128, 12], F32, tag="bias0")
                nc.sync.dma_start(bias0_sb[:], bias0_d.ap())
                brz1_sb = res.tile([128, 8, BL], F32, tag="brz1")
                nc.sync.dma_start(brz1_sb[:], brz1_d.ap())
                bnh0_sb = res.tile([128, 4, BL], F32, tag="bnh0")
                nc.sync.dma_start(bnh0_sb[:], bnh0_d.ap())
                bni1_sb = res.tile([128, 4, BL], F32, tag="bni1")
                nc.sync.dma_start(bni1_sb[:], bni1_d.ap())
                bnh1_sb = res.tile([128, 4, BL], F32, tag="bnh1")
                nc.sync.dma_start(bnh1_sb[:], bnh1_d.ap())

            NPRE = 2
            wpre_sb = [res.tile([128, 8, 8, 128], BF16, tag=f"wpre{i}",
                                name=f"wpre{i}")
                       for i in range(NPRE)]
            for i in range(NPRE):
                nc.sync.dma_start(wpre_sb[i][:], woutT_d.ap()[i])

            encP_sb = res.tile([128, 4, BL, T], BF16, tag="encP")
            embW0_sb = res.tile([128, 12, U, BL], F32, tag="embW0")
            yT_sb = res.tile([128, 8, U, BL], BF16, tag="yT")

            # persistent recurrence state (h kept in bf16)
            h0b = res.tile([128, 4, BL], BF16, tag="h0b")
            zero_sb = res.tile([128, 4, BL], BF16, tag="zero")
            nc.gpsimd.memset(h0b[:], 0.0)
            nc.gpsimd.memset(zero_sb[:], 0.0)

            # ---- setup: encP = W_attn^T-projected enc; embW0 = Wih0_emb @ emb ----
            with (
                tc.tile_pool(name="su", bufs=1) as su,
                tc.tile_pool(name="sup", bufs=2, space="PSUM") as sup,
            ):
                encE_sb = su.tile([128, 4, BL, T], BF16, tag="encE")
                nc.sync.dma_start(encE_sb[:], encE_d.ap())
                wattnT_sb = su.tile([128, 4, 4, 128], BF16, tag="wattnT")
                nc.sync.dma_start(wattnT_sb[:], wattnT_d.ap())
                wemb0_sb = su.tile([128, 4, 12, 128], BF16, tag="wemb0")
                nc.sync.dma_start(wemb0_sb[:], wemb0_d.ap())

                for m in range(12):
                    ew_ps = sup.tile([128, U * BL], F32, tag="ewps")
                    for kc in range(4):
                        nc.tensor.matmul(
                            ew_ps[:],
                            wemb0_sb[:, kc, m, :],
                            embT_sb[:, kc, :, :],
                            start=(kc == 0), stop=(kc == 3),
                        )
                    if biases_zero:
                        if m % 2 == 0:
                            nc.vector.tensor_copy(embW0_sb[:, m, :, :], ew_ps[:])
                        else:
                            nc.scalar.copy(embW0_sb[:, m, :, :], ew_ps[:])
                    else:
                        nc.scalar.activation(embW0_sb[:, m, :, :], ew_ps[:],
                                             ACTF.Identity, bias=bias0_sb[:, m:m + 1])

                for b in range(BL):
                    for hc in range(4):
                        ep_ps = sup.tile([128, T], F32, tag="epps")
                        for ec in range(4):
                            nc.tensor.matmul(
                                ep_ps[:],
                                wattnT_sb[:, ec, hc, :],
                                encE_sb[:, ec, b, :],
                                start=(ec == 0), stop=(ec == 3),
                            )
                        if (b + hc) % 2 == 0:
                            nc.vector.tensor_copy(encP_sb[:, hc, b, :], ep_ps[:])
                        else:
                            nc.scalar.copy(encP_sb[:, hc, b, :], ep_ps[:])

            # ---- phase 1: recurrence ----
            with (
                tc.tile_pool(name="p1s", bufs=1) as p1s,
                tc.tile_pool(name="p1p", bufs=1, space="PSUM") as p1p,
            ):
                # persistent PSUM tiles; scores rotate on u parity for the
                # mask pre-accumulation trick
                sc_ps = [p1p.tile([128, T], F32, tag=f"sc{i}", name=f"sc{i}")
                         for i in range(2)]
                tT_ps = p1p.tile([128, 8, 128], BF16, tag="tT")
                ctx_ps = p1p.tile([128, E], F32, tag="ctx")
                rz0_ps = p1p.tile([128, 8, BL], F32, tag="rz0")
                nih0_ps = p1p.tile([128, 8, BL], F32, tag="nih0")
                rz1_ps = p1p.tile([128, 8, BL], F32, tag="rz1")
                nih1_ps = p1p.tile([128, 8, BL], F32, tag="nih1")

                att_sb = p1s.tile([128, T], BF16, tag="att")
                ssum = p1s.tile([128, 1], F32, tag="ssum")
                rec = p1s.tile([128, 1], F32, tag="rec")
                atT_sb = p1s.tile([128, 4, BL], BF16, tag="atTsb")
                ctx_sb = p1s.tile([128, E], BF16, tag="ctxsb")
                g0_sb = p1s.tile([128, 8, BL], F32, tag="g0")
                t0_sb = p1s.tile([128, 8, BL], F32, tag="t0")
                ni0_sb = p1s.tile([128, 4, BL], F32, tag="ni0")
                a0_sb = p1s.tile([128, 4, BL], F32, tag="a0")
                np0_sb = p1s.tile([128, 4, BL], F32, tag="np0")
                n0_sb = p1s.tile([128, 4, BL], F32, tag="n0")
                d0_sb = p1s.tile([128, 4, BL], F32, tag="d0")
                g1_sb = p1s.tile([128, 8, BL], F32, tag="g1")
                t1_sb = p1s.tile([128, 8, BL], F32, tag="t1")
                a1_sb = p1s.tile([128, 4, BL], F32, tag="a1")
                np1_sb = p1s.tile([128, 4, BL], F32, tag="np1")
                n1_sb = p1s.tile([128, 4, BL], F32, tag="n1")
                d1_sb = p1s.tile([128, 4, BL], F32, tag="d1")
                ones_sb = p1s.tile([1, 1], BF16, tag="ones")
                ones128_sb = p1s.tile([1, 128], BF16, tag="ones128")
                zrow_sb = p1s.tile([1, T], BF16, tag="zrow")
                nc.gpsimd.memset(ones_sb[:], 1.0)
                nc.gpsimd.memset(ones128_sb[:], 1.0)
                nc.gpsimd.memset(zrow_sb[:], 0.0)

                # one-time init: write every partition row of the score/ctx
                # banks so never-again-written rows hold 0, not pre-kernel
                # garbage (exp/transpose would otherwise see inf/NaN there).
                for i in range(2):
                    nc.tensor.matmul(
                        sc_ps[i][:, :], ones128_sb[:], zrow_sb[:],
                        start=True, stop=False, skip_group_check=True,
                    )
                nc.tensor.matmul(
                    ctx_ps[:, :], ones128_sb[:], zrow_sb[:, 0:E],
                    start=True, stop=False, skip_group_check=True,
                )

                # mask pre-accumulation for u=0
                for b in range(BL):
                    nc.tensor.matmul(
                        sc_ps[0][32 * b:32 * b + 1, :],
                        ones_sb[:], mask_sb[:, b, :],
                        start=True, stop=False, skip_group_check=True,
                        tile_position=(0, 32 * b),
                    )

                def mm_nh(gps, wsb, rhs_fn, u):
                    for m in range(4):
                        for k in range(4):
                            nc.tensor.matmul(
                                gps[:, 4 + m, :], wsb[:, k, m, :], rhs_fn(k),
                                start=(k == 0), stop=(k == 3),
                            )

                def mm_rz_h(gps, wsb, rhs_fn, u, ms=range(8)):
                    for m in ms:
                        for k in range(4):
                            nc.tensor.matmul(
                                gps[:, m, :], wsb[:, k, m, :], rhs_fn(k),
                                start=(k == 0), stop=False,
                            )

                def h0rhs_fn(u):
                    return (lambda k: zero_sb[:, k, :]) if u == 0 else \
                           (lambda k: h0b[:, k, :])

                def h1rhs_fn(u):
                    return (lambda k: zero_sb[:, k, :]) if u == 0 else \
                           (lambda k: yT_sb[:, k, u - 1, :])

                # u=0 h-dependent GRU0 contractions (zeros)
                mm_nh(nih0_ps, wn0h_sb, h0rhs_fn(0), 0)
                mm_rz_h(rz0_ps, wrz0_sb, h0rhs_fn(0), 0)

                for u in range(u_steps):
                    cur = sc_ps[u % 2]
                    nxt = sc_ps[(u + 1) % 2]

                    # scores[b, t] += sum_h q[h, b] * encP[b][h, t]
                    for b in range(BL):
                        for kc in range(4):
                            lhs = (embT_sb[:, kc, 0, b:b + 1] if u == 0
                                   else yT_sb[:, kc, u - 1, b:b + 1])
                            nc.tensor.matmul(
                                cur[32 * b:32 * b + 1, :],
                                lhs,
                                encP_sb[:, kc, b, :],
                                start=False, stop=(kc == 3),
                                skip_group_check=True,
                                tile_position=(0, 32 * b),
                            )

                    # GRU1 h1-dependent contractions fill the softmax gap
                    mm_nh(nih1_ps, wn1h_sb, h1rhs_fn(u), u)
                    mm_rz_h(rz1_ps, wrz1_sb, h1rhs_fn(u), u, ms=range(4))

                    # softmax (no max-subtract; mask rows are -1e30)
                    nc.scalar.activation(att_sb[:], cur[:], ACTF.Exp,
                                         accum_out=ssum[:])
                    nc.vector.reciprocal(rec[:], ssum[:])

                    # attT: batch b sits in column 32b; keep those columns
                    for tc4 in range(4):
                        nc.tensor.transpose(
                            tT_ps[:, tc4, :],
                            att_sb[:, tc4 * 128:(tc4 + 1) * 128],
                            identb_sb[:],
                        )
                    nc.vector.tensor_copy(atT_sb[:], tT_ps[:, 0:4, 0:128:32])

                    # ctx[b, e] += att[b, t] * encT[b][t, e]
                    for b in range(BL):
                        for tc4 in range(4):
                            nc.tensor.matmul(
                                ctx_ps[32 * b:32 * b + 1, :],
                                atT_sb[:, tc4, b:b + 1],
                                encT_sb[:, tc4, b, :],
                                start=(tc4 == 0), stop=(tc4 == 3),
                                skip_group_check=True,
                                tile_position=(0, 32 * b),
                            )

                    # ctx normalize-on-copy (scale = 1/sum per batch row),
                    # split across scalar+vector engines
                    nc.vector.tensor_scalar_mul(ctx_sb[:, 0:384],
                                                ctx_ps[:, 0:384], rec[:])
                    nc.scalar.activation(ctx_sb[:, 384:E],
                                         ctx_ps[:, 384:E], ACTF.Copy,
                                         scale=rec[:])
                    # ctxT transposes then GRU0 ctx contractions
                    for ec in range(4):
                        nc.tensor.transpose(
                            tT_ps[:, 4 + ec, :],
                            ctx_sb[:, ec * 128:(ec + 1) * 128],
                            identb_sb[:],
                        )
                    nc.vector.tensor_copy(yT_sb[:, 4:8, u, :],
                                          tT_ps[:, 4:8, 0:128:32])
                    for m in range(8):
                        for k in range(4):
                            nc.tensor.matmul(
                                rz0_ps[:, m, :],
                                wrz0_sb[:, 4 + k, m, :],
                                yT_sb[:, 4 + k, u, :],
                                start=False, stop=(k == 3),
                            )
                    for m in range(4):
                        for k in range(4):
                            nc.tensor.matmul(
                                nih0_ps[:, m, :],
                                wn0i_sb[:, k, m, :],
                                yT_sb[:, 4 + k, u, :],
                                start=(k == 0), stop=(k == 3),
                            )

                    # mask pre-accumulation for u+1 fills the gate0 gap
                    if u + 1 < u_steps:
                        for b in range(BL):
                            nc.tensor.matmul(
                                nxt[32 * b:32 * b + 1, :],
                                ones_sb[:], mask_sb[:, b, :],
                                start=True, stop=False, skip_group_check=True,
                                tile_position=(0, 32 * b),
                            )

                    # ---- GRU0 gate math ([128, m, b] layout) ----
                    nc.vector.tensor_tensor(g0_sb[:], rz0_ps[:],
                                            embW0_sb[:, 0:8, u, :], op=ALU.add)
                    nc.scalar.activation(t0_sb[:], g0_sb[:], ACTF.Tanh,
                                         scale=0.5)
                    nc.vector.tensor_tensor(ni0_sb[:], nih0_ps[:, 0:4, :],
                                            embW0_sb[:, 8:12, u, :], op=ALU.add)
                    if biases_zero:
                        nc.vector.scalar_tensor_tensor(
                            a0_sb[:], t0_sb[:, 0:4, :], 1.0,
                            nih0_ps[:, 4:8, :],
                            op0=ALU.add, op1=ALU.mult)
                    else:
                        nc.vector.tensor_tensor(a0_sb[:],
                                                nih0_ps[:, 4:8, :],
                                                bnh0_sb[:], op=ALU.add)
                        nc.vector.scalar_tensor_tensor(
                            a0_sb[:], t0_sb[:, 0:4, :], 1.0, a0_sb[:],
                            op0=ALU.add, op1=ALU.mult)
                    nc.vector.scalar_tensor_tensor(
                        np0_sb[:], a0_sb[:], 0.5, ni0_sb[:],
                        op0=ALU.mult, op1=ALU.add)
                    nc.scalar.activation(n0_sb[:], np0_sb[:], ACTF.Tanh)
                    nc.vector.tensor_tensor(d0_sb[:], h0b[:], n0_sb[:],
                                            op=ALU.subtract)
                    nc.vector.scalar_tensor_tensor(
                        d0_sb[:], t0_sb[:, 4:8, :], 1.0, d0_sb[:],
                        op0=ALU.add, op1=ALU.mult)
                    nc.vector.scalar_tensor_tensor(
                        h0b[:], d0_sb[:], 0.5, n0_sb[:],
                        op0=ALU.mult, op1=ALU.add)

                    # GRU1 h0n-dependent contractions
                    for m in range(8):
                        for k in range(4):
                            nc.tensor.matmul(
                                rz1_ps[:, m, :],
                                wrz1_sb[:, 4 + k, m, :],
                                h0b[:, k, :],
                                start=False, stop=(k == 3),
                            )
                    for m in range(4):
                        for k in range(4):
                            nc.tensor.matmul(
                                nih1_ps[:, m, :],
                                wn1i_sb[:, k, m, :],
                                h0b[:, k, :],
                                start=(k == 0), stop=(k == 3),
                            )

                    # next step's h0-dependent GRU0 contractions fill the
                    # GRU1 gate-math gap
                    if u + 1 < u_steps:
                        mm_nh(nih0_ps, wn0h_sb, h0rhs_fn(u + 1), u + 1)
                        mm_rz_h(rz0_ps, wrz0_sb, h0rhs_fn(u + 1), u + 1)

                    # ---- GRU1 gate math ----
                    if biases_zero:
                        nc.scalar.activation(t1_sb[:], rz1_ps[:],
                                             ACTF.Tanh, scale=0.5)
                        nc.vector.scalar_tensor_tensor(
                            a1_sb[:], t1_sb[:, 0:4, :], 1.0,
                            nih1_ps[:, 4:8, :],
                            op0=ALU.add, op1=ALU.mult)
                        nc.vector.scalar_tensor_tensor(
                            np1_sb[:], a1_sb[:], 0.5, nih1_ps[:, 0:4, :],
                            op0=ALU.mult, op1=ALU.add)
                    else:
                        nc.vector.tensor_tensor(g1_sb[:], rz1_ps[:],
                                                brz1_sb[:], op=ALU.add)
                        nc.scalar.activation(t1_sb[:], g1_sb[:], ACTF.Tanh,
                                             scale=0.5)
                        nc.vector.tensor_tensor(a1_sb[:],
                                                nih1_ps[:, 4:8, :],
                                                bnh1_sb[:], op=ALU.add)
                        nc.vector.scalar_tensor_tensor(
                            a1_sb[:], t1_sb[:, 0:4, :], 1.0, a1_sb[:],
                            op0=ALU.add, op1=ALU.mult)
                        nc.vector.tensor_tensor(np1_sb[:],
                                                nih1_ps[:, 0:4, :],
                                                bni1_sb[:], op=ALU.add)
                        nc.vector.scalar_tensor_tensor(
                            np1_sb[:], a1_sb[:], 0.5, np1_sb[:],
                            op0=ALU.mult, op1=ALU.add)
                    nc.scalar.activation(n1_sb[:], np1_sb[:], ACTF.Tanh)
                    d1_rhs = (zero_sb[:, :, :] if u == 0
                              else yT_sb[:, 0:4, u - 1, :])
                    nc.vector.tensor_tensor(d1_sb[:], d1_rhs, n1_sb[:],
                                            op=ALU.subtract)
                    nc.vector.scalar_tensor_tensor(
                        d1_sb[:], t1_sb[:, 4:8, :], 1.0, d1_sb[:],
                        op0=ALU.add, op1=ALU.mult)
                    nc.vector.scalar_tensor_tensor(
                        yT_sb[:, 0:4, u, :], d1_sb[:], 0.5, n1_sb[:],
                        op0=ALU.mult, op1=ALU.add)

            # ---- phase 2: full-vocab projection for the local batches ----
            with (
                tc.tile_pool(name="p2w", bufs=2) as p2w,
                tc.tile_pool(name="p2o", bufs=2) as p2o,
                tc.tile_pool(name="p2p", bufs=4, space="PSUM") as p2p,
            ):
                for s in range(NSUP):
                    if s < NPRE:
                        wt = wpre_sb[s]
                    else:
                        wt = p2w.tile([128, 8, 8, 128], BF16, tag="wt")
                        nc.sync.dma_start(wt[:], woutT_d.ap()[s])
                    ob = p2o.tile([128, 8, UB_L], BF16, tag="ob")
                    for vc in range(8):
                        ps = p2p.tile([128, UB_L], F32, tag="p2")
                        for kc in range(8):
                            nc.tensor.matmul(
                                ps[:],
                                wt[:, vc, kc, :],
                                yT_sb[:, kc, :, :],
                                start=(kc == 0), stop=(kc == 7),
                            )
                        if vc % 2 == 0:
                            nc.scalar.activation(
                                ob[:, vc, :], ps[:], ACTF.Identity,
                                bias=bout_sb[:, s * 8 + vc:s * 8 + vc + 1])
                        else:
                            nc.vector.tensor_scalar_add(
                                ob[:, vc, :], ps[:],
                                bout_sb[:, s * 8 + vc:s * 8 + vc + 1])
                    nc.sync.dma_start(out_d.ap()[s], ob[:])

    nc.finalize()
    return nc


_NC_CACHE = {}


def _get_nc(biases_zero=True):
    if biases_zero not in _NC_CACHE:
        _NC_CACHE[biases_zero] = build_nc(biases_zero=biases_zero)
    return _NC_CACHE[biases_zero]


def make_in_maps(inputs):
    f32 = np.float32
    bf = ml_dtypes.bfloat16
    enc = np.asarray(inputs["encoder_out"], f32)
    lens = np.asarray(inputs["encoder_lens"]).astype(np.int64)
    dec = np.asarray(inputs["decoder_in"]).astype(np.int64)
    emb_table = np.asarray(inputs["emb_table"], f32)
    W_attn = np.asarray(inputs["W_attn"], f32)
    W_ih0 = np.asarray(inputs["W_ih0"], f32)
    W_hh0 = np.asarray(inputs["W_hh0"], f32)
    b_ih0 = np.asarray(inputs["b_ih0"], f32)
    b_hh0 = np.asarray(inputs["b_hh0"], f32)
    W_ih1 = np.asarray(inputs["W_ih1"], f32)
    W_hh1 = np.asarray(inputs["W_hh1"], f32)
    b_ih1 = np.asarray(inputs["b_ih1"], f32)
    b_hh1 = np.asarray(inputs["b_hh1"], f32)
    W_out = np.asarray(inputs["W_out"], f32)
    b_out = np.asarray(inputs["b_out"], f32)

    embedded = emb_table[dec]                       # [B, U, H]
    mask = np.where(
        np.arange(T)[None, :] >= lens[:, None],
        f32(-1e30), f32(0.0))                       # [B, T]

    def chunkT(w):
        # [K, M] weight -> lhsT chunks [128, kc, mc, 128] (bf16)
        K, M = w.shape
        return np.ascontiguousarray(
            w.reshape(K // 128, 128, M // 128, 128).transpose(1, 0, 2, 3)
        ).astype(bf)

    # per-step GRU lhsT chunk tables; k-order: h-part first, then ctx/x-part
    wrz0 = np.concatenate([W_hh0[0:1024].T, W_ih0[0:1024, 512:1024].T], 0)
    wrz0 = chunkT(wrz0)                             # [128, 8, 8, 128]
    wn0i = chunkT(W_ih0[1024:1536, 512:1024].T)
    wn0h = chunkT(W_hh0[1024:1536].T)
    wrz1 = np.concatenate([W_hh1[0:1024].T, W_ih1[0:1024].T], 0)
    wrz1 = chunkT(wrz1)
    wn1i = chunkT(W_ih1[1024:1536].T)
    wn1h = chunkT(W_hh1[1024:1536].T)
    wemb0 = chunkT(W_ih0[:, 0:512].T)               # [128, 4, 12, 128]
    wattnT = chunkT(W_attn.T)                       # [128, 4ec, 4hc, 128]

    Wp = np.zeros((VP, 1024), f32)
    Wp[:V] = W_out
    woutT = np.ascontiguousarray(
        Wp.reshape(NSUP, 8, 128, 8, 128).transpose(0, 4, 1, 3, 2)
    ).astype(bf)                                    # [32, 128k, 8vc, 8kc, 128v]
    bp = np.zeros((VP,), f32)
    bp[:V] = b_out
    bout_t = np.ascontiguousarray(bp.reshape(NSUP * 8, 128).T)

    # biases
    bias0 = np.zeros((128, 12), f32)                # embW0 bias (rz: ih+hh, n_i: ih)
    brz = (b_ih0[:1024] + b_hh0[:1024]).reshape(8, 128).T
    bias0[:, 0:8] = brz
    bias0[:, 8:12] = b_ih0[1024:1536].reshape(4, 128).T
    bcast = lambda v: np.ascontiguousarray(np.broadcast_to(
        v.reshape(v.shape[0] // 128, 128).T[:, :, None], (128, v.shape[0] // 128, BL)))
    brz1 = bcast(b_ih1[:1024] + b_hh1[:1024])
    bnh0 = bcast(b_hh0[1024:1536])
    bni1 = bcast(b_ih1[1024:1536])
    bnh1 = bcast(b_hh1[1024:1536])

    identb = np.eye(128, dtype=f32).astype(bf)

    in_maps = []
    for c in range(NCORES):
        bs = slice(BL * c, BL * (c + 1))
        encl = enc[bs]                              # [BL, T, E]
        encE = np.ascontiguousarray(
            encl.transpose(2, 0, 1).reshape(4, 128, BL, T).transpose(1, 0, 2, 3)
        ).astype(bf)                                # [128, 4ec, BL, T]
        encTt = np.ascontiguousarray(
            encl.transpose(1, 0, 2).reshape(4, 128, BL, E).transpose(1, 0, 2, 3)
        ).astype(bf)                                # [128, 4tc, BL, E]
        embT = np.ascontiguousarray(
            embedded[bs].transpose(2, 1, 0).reshape(4, 128, U, BL).transpose(1, 0, 2, 3)
        ).astype(bf)                                # [128, 4hc, U, BL]
        in_maps.append({
            "encE": encE,
            "encT": encTt,
            "embT": embT,
            "mask": np.ascontiguousarray(mask[bs][None, :, :]).astype(bf),
            "wattnT": wattnT,
            "wemb0": wemb0,
            "wrz0": wrz0, "wn0i": wn0i, "wn0h": wn0h,
            "wrz1": wrz1, "wn1i": wn1i, "wn1h": wn1h,
            "woutT": woutT,
            "bout": bout_t,
            "identb": identb,
            "bias0": bias0,
            "brz1": brz1, "bnh0": bnh0, "bni1": bni1, "bnh1": bnh1,
        })
    return in_maps


def assemble_output(results):
    logits = np.zeros((B, U, V), np.float32)
    for c in range(NCORES):
        o = np.asarray(results[c]["out"], np.float32)  # [32, 128, 8, U, BL]
        o = o.transpose(4, 3, 0, 2, 1).reshape(BL, U, VP)
        logits[BL * c:BL * (c + 1)] = o[:, :, :V]
    return logits


def kernel(**inputs):
    bz = all(
        float(np.abs(np.asarray(inputs[k])).max()) == 0.0
        for k in ("b_ih0", "b_hh0", "b_ih1", "b_hh1")
    )
    nc = _get_nc(biases_zero=bz)
    in_maps = make_in_maps(inputs)
    res = run_bass_kernel_spmd(nc, in_maps, core_ids=list(range(NCORES)))
    return assemble_output(res.results)


if __name__ == "__main__":
    nc = build_nc()
    print("built OK")
